# revision 1
# baseline (speedup 1.0000x reference)
"""Trainium2 Bass kernel for nn_ExpertsLinear (weighted mixture of 8 experts).

    y[b, o] = sum_e weights[b, e] * (x @ W[e] + b[e])[b, o]

Full shapes: x [65536, 512] f32, weights [65536, 8] f32,
W [8, 512, 512] f32, b [8, 1, 512] f32 -> y [65536, 512] f32.

Sharding: data-parallel over batch across 8 NeuronCores (8192 rows each);
W replicated. The bias term (always zero in this problem's inputs) is
applied host-side only if nonzero.

Per-core kernel, per 128-row batch tile:
  - DMA x tile f32 -> SBUF, cast to fp16 (DVE)
  - transpose to xT [128 feat, 4, 128 b] via PE transpose-mode
  - per expert e: z_e = sum_fc xT[:, fc, :].T @ W16[e, fc]  (PSUM fp32,
    one bank per expert, fp16 inputs)
  - combine on ACT/DVE: y = sum_e weights[:, e] * z_e  (per-partition
    scalar scale), store fp32.
"""

import numpy as np

P = 128
D = 512
E = 8
FC = D // P
N_CORES = 8
B_FULL = 65536
B_LOC = B_FULL // N_CORES

_COMPILED = {}


def _build_nc(transpose_mode="pe"):
    import concourse.bacc as bacc
    import concourse.mybir as mybir
    import concourse.tile as tile
    from concourse.masks import make_identity

    F32 = mybir.dt.float32
    F16 = mybir.dt.float16

    nc = bacc.Bacc(
        "TRN2",
        target_bir_lowering=False,
        debug=False,
        enable_asserts=False,
        num_devices=N_CORES,
    )
    x_d = nc.dram_tensor("x", [B_LOC, D], F32, kind="ExternalInput").ap()
    w_d = nc.dram_tensor("weights", [B_LOC, E], F32, kind="ExternalInput").ap()
    W_d = nc.dram_tensor("W", [E, D, D], F32, kind="ExternalInput").ap()
    y_d = nc.dram_tensor("y", [B_LOC, D], F32, kind="ExternalOutput").ap()

    nbt = B_LOC // P

    with tile.TileContext(nc) as tc:
        with (
            tc.tile_pool(name="const", bufs=1) as const_pool,
            tc.tile_pool(name="xf32", bufs=3) as xf_pool,
            tc.tile_pool(name="xh16", bufs=3) as xh_pool,
            tc.tile_pool(name="xT16", bufs=3) as xT_pool,
            tc.tile_pool(name="tpsum", bufs=2, space="PSUM") as tp_pool,
            tc.tile_pool(name="zpsum", bufs=6, space="PSUM") as z_pool,
            tc.tile_pool(name="tmul", bufs=3) as t_pool,
            tc.tile_pool(name="yout", bufs=3) as y_pool,
        ):
            W_sb = const_pool.tile([P, E, FC, D], F16, name="W_sb")
            for e in range(E):
                for fc in range(FC):
                    nc.gpsimd.dma_start(
                        out=W_sb[:, e, fc, :],
                        in_=W_d[e, fc * P : (fc + 1) * P, :],
                    )

            w_sb = const_pool.tile([P, nbt, E], F32, name="w_sb")
            nc.sync.dma_start(out=w_sb[:], in_=w_d.rearrange("(t p) e -> p t e", p=P))

            if transpose_mode == "pe":
                ident = const_pool.tile([P, P], F16, name="ident")
                make_identity(nc, ident)

            for bt in range(nbt):
                xf = xf_pool.tile([P, D], F32, name="xf")
                nc.sync.dma_start(out=xf[:], in_=x_d[bt * P : (bt + 1) * P, :])
                xh = xh_pool.tile([P, D], F16, name="xh")
                nc.vector.tensor_copy(out=xh[:], in_=xf[:])

                xT = xT_pool.tile([P, FC, P], F16, name="xT")
                if transpose_mode == "pe":
                    tp = tp_pool.tile([P, FC, P], F16, name="tp")
                    for fc in range(FC):
                        nc.tensor.transpose(
                            tp[:, fc, :], xh[:, fc * P : (fc + 1) * P], ident[:]
                        )
                    nc.vector.tensor_copy(out=xT[:], in_=tp[:])
                else:
                    nc.sync.dma_start_transpose(xT[:], xh[:])

                z_tiles = [None] * E
                for half in range(2):
                    for fc in range(FC):
                        lhsT = xT[:, fc, :]
                        for ei in range(4):
                            e = half * 4 + ei
                            if fc == 0:
                                z_tiles[e] = z_pool.tile([P, D], F32, name="z")
                            nc.tensor.matmul(
                                z_tiles[e][:],
                                lhsT=lhsT,
                                rhs=W_sb[:, e, fc, :],
                                start=(fc == 0),
                                stop=(fc == FC - 1),
                            )

                y_t = y_pool.tile([P, D], F32, name="y_t")
                nc.scalar.mul(y_t[:], z_tiles[0][:], w_sb[:, bt, 0:1])
                for e in range(1, E):
                    t_e = t_pool.tile([P, D], F32, name="t_e")
                    nc.scalar.mul(t_e[:], z_tiles[e][:], w_sb[:, bt, e : e + 1])
                    nc.vector.tensor_add(out=y_t[:], in0=y_t[:], in1=t_e[:])

                nc.sync.dma_start(out=y_d[bt * P : (bt + 1) * P, :], in_=y_t[:])

    nc.compile()
    return nc


def _get_nc():
    if "nc" not in _COMPILED:
        _COMPILED["nc"] = _build_nc()
    return _COMPILED["nc"]


def kernel(x, weights, W, b):
    from concourse.bass_utils import run_bass_kernel_spmd

    x = np.ascontiguousarray(np.asarray(x, dtype=np.float32))
    weights = np.ascontiguousarray(np.asarray(weights, dtype=np.float32))
    W_np = np.ascontiguousarray(np.asarray(W, dtype=np.float32))
    b_np = np.asarray(b, dtype=np.float32)

    nc = _get_nc()

    xs = x.reshape(N_CORES, B_LOC, D)
    ws = weights.reshape(N_CORES, B_LOC, E)
    in_maps = [
        {"x": xs[c], "weights": ws[c], "W": W_np} for c in range(N_CORES)
    ]
    res = run_bass_kernel_spmd(nc, in_maps, core_ids=list(range(N_CORES)))
    y = np.concatenate([res.results[c]["y"] for c in range(N_CORES)], axis=0)

    # Bias term (zero for this problem's inputs; handled host-side for
    # exactness if ever nonzero).
    if np.any(b_np):
        y = y + weights @ b_np[:, 0, :]

    return y.astype(np.float32)
